# revision 5
# baseline (speedup 1.0000x reference)
"""BitFeedForward (BitNet b1.58 FFN) on 8 Trainium2 NeuronCores — fp8 DoubleRow.

Data-parallel over tokens (1024/core, weights replicated). All quantized
matmul operands are exact in fp8-e4m3 via:
  - weights: 128*clip(round(w*s_w),-1,1) in {-128,0,128} (128 = 2^7 folds out
    of the drain exactly)
  - activations: integer split a = hi16 + lo with hi16 = round16(a) (multiple
    of 16, |.|<=128) and lo = a - hi16 in [-8,8]; both e4m3-exact.
One DoubleRow matmul (0.5 cyc/row, 2 planes) then computes
  sum_i st[:,i].T @ mv[:,i] = 128*wq^T(hi16+lo) = 128*wq^T a    -- exact --
at 2x bf16 throughput; weights pass a zero-stride broadcast AP as both planes.

Schedule per core:
  S:  w-mean shard sums -> AllGather -> scales (as before)
  X:  per-token absmax quant -> xqT (PE/DMA transpose) -> (hi16,lo) pair tile
  L1: stream w1t f32 (2048-wide slabs) -> 3-pass ternarize (ACT/Pool/DVE)
      into an e4m3 quarter-ring -> DoubleRow matmuls -> relu int16 h to HBM
  M:  cross-partition max via PE transpose -> per-token g / fscale
  H:  h int16 back (8-ob groups) -> (hih16,loh) e4m3 pair tile (resident)
  L2: mg-outer, JIT 4-ob-group w2 ternarize, DoubleRow matmuls, scaled drains
"""
import os
import numpy as np

import concourse.bass as bass
import concourse.mybir as mybir
import concourse.tile as tile
from concourse.bass_utils import run_bass_kernel_spmd
from concourse.masks import make_identity

F32 = mybir.dt.float32
BF16 = mybir.dt.bfloat16
E4 = mybir.dt.float8e4
I16 = mybir.dt.int16
AX = mybir.AxisListType
OP = mybir.AluOpType
ACTF = mybir.ActivationFunctionType
PM = mybir.MatmulPerfMode

NCORES = 8
MAGIC = 12582912.0           # 1.5*2^23: f32 round-to-int
MAGIC16 = 201326592.0        # 1.5*2^27: f32 round-to-multiple-of-16
BMAGIC = 384.0               # 1.5*2^8: bf16 round-to-int (output-convert)
EPS = 1e-5
B, S, D = 4, 2048, 2048
O = 8192
T = (B * S) // NCORES        # 1024 tokens per core
INV_NW = float(np.float32(1.0 / (O * D)))

KB = D // 128                # 16 k-blocks (L1 contraction)
OB = O // 128                # 64 o-blocks (L2 contraction)
NG1 = O // 512               # 16 L1 generations
TB = T // 128                # 8 token blocks
TC = T // 512                # 2 token chunks


def _split_excess_waits(nc, max_waits=1):
    """Walrus rejects >1 sync wait per instruction; move extras onto NoOps."""
    for fn in nc.m.functions:
        for blk in fn.blocks:
            out = []
            for inst in blk.instructions:
                si = inst.sync_info
                waits = list(si.on_wait) if si is not None and si.on_wait else []
                if len(waits) > max_waits:
                    extra, keep = waits[:-max_waits], waits[-max_waits:]
                    for i in range(0, len(extra), max_waits):
                        out.append(mybir.InstNoOp(
                            name=f"{inst.name}-wsplit{i}",
                            sync_info=mybir.SyncInfo(
                                on_wait=extra[i:i + max_waits], on_update=[]),
                            bass_nofuse=True,
                            engine=inst.engine,
                        ))
                    si.on_wait = keep
                out.append(inst)
            try:
                blk.instructions = out
            except Exception:
                blk.instructions.clear()
                blk.instructions.extend(out)


def build_nc():
    nc = bass.Bass()
    x_in = nc.dram_tensor("x", [T, D], F32, kind="ExternalInput")
    w1t_in = nc.dram_tensor("w1t", [D, O], F32, kind="ExternalInput")
    w2t_in = nc.dram_tensor("w2t", [O, D], F32, kind="ExternalInput")
    wsh1 = nc.dram_tensor("wsh1", [256, 8192], F32, kind="ExternalInput")
    wsh2 = nc.dram_tensor("wsh2", [1024, 2048], F32, kind="ExternalInput")
    y_out = nc.dram_tensor("out", [T, D], F32, kind="ExternalOutput")

    with tile.TileContext(nc) as tc:
        with tc.tile_pool(name="const", bufs=1) as cp, \
             tc.tile_pool(name="dram", bufs=1, space="DRAM") as dram:

            ident = cp.tile([128, 128], F32)
            make_identity(nc, ident[:])
            ones_row = cp.tile([1, 128], F32)
            nc.vector.memset(ones_row[:], 1.0)
            ones_col = cp.tile([128, 1], F32)
            ident_bf = cp.tile([128, 128], BF16)
            nc.vector.memset(ones_col[:], 1.0)
            C = cp.tile([128, TB], F32)       # c[t] = max_x[t]*mu1/127
            fscale = cp.tile([128, TB], F32)  # hmaxc[t]*mu2/127/128
            G = cp.tile([128, T], mybir.dt.float16)  # g[t] bcast (clamped)

            h_hbm = dram.tile([O, T], I16)

            # ==== L1 era =====================================================
            with tc.tile_pool(name="l1big", bufs=1) as l1p:
                xpair = l1p.tile([128, KB, 2, T], E4)
                macc = l1p.tile([128, T], I16)
                nc.vector.memset(macc[:], 0.0)
                xq_dram = dram.tile([T, D], BF16)

                with tc.tile_pool(name="ps_s", bufs=2, space="PSUM") as pss, \
                     tc.tile_pool(name="mu", bufs=2) as mup, \
                     tc.tile_pool(name="xio", bufs=2) as xp, \
                     tc.tile_pool(name="xqp", bufs=5) as xqp, \
                     tc.tile_pool(name="xqt", bufs=1) as xqtp, \
                     tc.tile_pool(name="xsc", bufs=2) as xsc:
                    xqT = xqtp.tile([128, KB, T], BF16)

                    chains = [[(wsh1, r * 128, c * 4096, 4096)
                               for r in range(2) for c in range(2)],
                              [(wsh2, r * 128, 0, 2048) for r in range(8)]]

                    def emit_mu_chain(j):
                        acc = cp.tile([128, 1], F32, name=f"acc{j}")
                        nc.vector.memset(acc[:], 0.0)
                        for (src, r0, c0, f) in chains[j]:
                            wt = mup.tile([128, 4096], F32, tag="mu")
                            nc.sync.dma_start(wt[:, :f], src[r0:r0 + 128, c0:c0 + f])
                            pr = mup.tile([128, 1], F32, tag="mupart")
                            nc.vector.tensor_reduce(pr[:], wt[:, :f], axis=AX.X,
                                                    op=OP.add,
                                                    apply_absolute_value=True)
                            nc.vector.tensor_tensor(acc[:], acc[:], pr[:], OP.add)
                        pss_t = pss.tile([1, 1], F32, tag="musum", name=f"musum{j}")
                        nc.tensor.matmul(pss_t[:], acc[:], ones_col[:],
                                         start=True, stop=True)
                        summ = cp.tile([1, 1], F32, name=f"sum{j}")
                        loc = mup.tile([1, 1], F32, tag="loc", name=f"loc{j}")
                        nc.scalar.copy(loc[:], pss_t[:])
                        cc_in = dram.tile([1, 1], F32, name=f"ccin{j}")
                        cc_out = dram.tile([NCORES, 1], F32,
                                           addr_space="Shared",
                                           name=f"ccout{j}")
                        nc.sync.dma_start(cc_in[:], loc[:])
                        nc.gpsimd.collective_compute(
                            "AllGather", OP.bypass,
                            replica_groups=[list(range(NCORES))],
                            ins=[cc_in[:].opt()], outs=[cc_out[:].opt()])
                        srow = mup.tile([1, NCORES], F32, tag="srow",
                                        name=f"srow{j}")
                        nc.sync.dma_start(srow[:],
                                          cc_out[:].rearrange("a b -> b a"))
                        nc.vector.tensor_reduce(summ[:], srow[:], axis=AX.X,
                                                op=OP.add)
                        muc = cp.tile([1, 1], F32, name=f"muc{j}")
                        nc.vector.tensor_scalar(muc[:], summ[:], INV_NW, EPS,
                                                OP.mult, OP.max)
                        vals = cp.tile([1, 2], F32, name=f"vals{j}")
                        nc.vector.reciprocal(vals[:, 0:1], muc[:])
                        nc.vector.tensor_scalar_mul(vals[:, 1:2], muc[:],
                                                    1.0 / 127.0)
                        psb = pss.tile([128, 2], F32, tag="bcast", name=f"bc{j}")
                        nc.tensor.matmul(psb[:], ones_row[:], vals[:],
                                         start=True, stop=True)
                        BCj = cp.tile([128, 2], F32, name=f"BC{j}")
                        nc.scalar.copy(BCj[:], psb[:])
                        return BCj

                    mc_all = cp.tile([128, TB], F32, name="mc_all")

                    def emit_x_tile(xb):
                        xt = xp.tile([128, D], F32, tag="xload")
                        nc.sync.dma_start(xt[:], x_in[xb * 128:(xb + 1) * 128, :])
                        mr = xsc.tile([128, 1], F32, tag="xmax")
                        nc.vector.tensor_reduce(mr[:], xt[:], axis=AX.X,
                                                op=OP.max,
                                                apply_absolute_value=True)
                        nc.vector.tensor_scalar_max(mc_all[:, xb:xb + 1],
                                                    mr[:], EPS)
                        rc = xsc.tile([128, 1], F32, tag="xrcp")
                        nc.vector.reciprocal(rc[:], mc_all[:, xb:xb + 1])
                        sx = xsc.tile([128, 1], F32, tag="xs")
                        nc.vector.tensor_scalar_mul(sx[:], rc[:], 127.0)
                        xr = xp.tile([128, D], F32, tag="xround")
                        nc.scalar.activation(xr[:], xt[:], ACTF.Copy,
                                             bias=MAGIC, scale=sx[:])
                        xq = xqp.tile([128, D], BF16, tag="xq")
                        nc.vector.tensor_scalar_add(xq[:], xr[:], -MAGIC)
                        nc.sync.dma_start(xq_dram[xb * 128:(xb + 1) * 128, :],
                                          xq[:])
                        return xq

                    make_identity(nc, ident_bf[:])
                    BC0 = emit_mu_chain(0)
                    mu1_127 = BC0[:, 1:2]
                    xqs0 = [emit_x_tile(xb) for xb in range(4)]
                    # chunk-0 transposes on the otherwise-idle PE
                    for tb4 in range(4):
                        for k in range(KB):
                            ptt = pss.tile([128, 128], BF16, tag="ptt",
                                           name=f"ptt{tb4}_{k}")
                            nc.tensor.transpose(
                                ptt[:], xqs0[tb4][:, k * 128:(k + 1) * 128],
                                ident_bf[:])
                            nc.vector.tensor_scalar_add(
                                xqT[:, k, tb4 * 128:(tb4 + 1) * 128], ptt[:],
                                0.0)
                    BC1 = emit_mu_chain(1)
                    for xb in range(4, TB):
                        emit_x_tile(xb)
                    nc.vector.tensor_scalar(C[:], mc_all[:], mu1_127, None,
                                            OP.mult)
                    # per-k transpose + pair build so L1 k-chains start early
                    for k in range(KB):
                        nc.sync.dma_start_transpose(
                            xqT[:, k, 512:1024],
                            xq_dram[512:1024, k * 128:(k + 1) * 128])
                        nc.vector.tensor_scalar(xpair[:, k, 0, :],
                                                xqT[:, k, :],
                                                MAGIC16, -MAGIC16,
                                                OP.add, OP.add)
                        nc.vector.tensor_tensor(xpair[:, k, 1, :],
                                                xqT[:, k, :],
                                                xpair[:, k, 0, :],
                                                OP.subtract)

                s_w1 = BC0[:, 0:1]
                s_w2 = BC1[:, 0:1]
                mu2_127 = BC1[:, 1:2]

                # ---- Phase L1: h = relu(int matmul)/256 -> int16, running max
                with tc.tile_pool(name="w1f", bufs=2) as wf, \
                     tc.tile_pool(name="w1b", bufs=2) as wbp, \
                     tc.tile_pool(name="wq1", bufs=2) as wqp, \
                     tc.tile_pool(name="hst", bufs=3) as hst, \
                     tc.tile_pool(name="ps1", bufs=4, space="PSUM") as ps1:
                    for q in range(4):
                        wq1 = wqp.tile([128, KB, 2048], E4, tag="wq1")
                        for kb in range(KB):
                            wsl = wf.tile([128, 2048], F32, tag="wf")
                            nc.sync.dma_start(
                                wsl[:], w1t_in[kb * 128:(kb + 1) * 128,
                                               q * 2048:(q + 1) * 2048])
                            t1 = wbp.tile([128, 2048], F32, tag="t1")
                            nc.scalar.activation(t1[:], wsl[:], ACTF.Copy,
                                                 bias=MAGIC, scale=s_w1)
                            t2 = wbp.tile([128, 2048], F32, tag="t2")
                            nc.vector.tensor_scalar(t2[:], t1[:],
                                                    MAGIC + 1.0, MAGIC - 1.0,
                                                    OP.min, OP.max)
                            nc.gpsimd.tensor_scalar(wq1[:, kb, :], t2[:],
                                                    -MAGIC, 128.0,
                                                    OP.add, OP.mult)
                        for g4 in range(4):
                            g = q * 4 + g4
                            pts = [ps1.tile([128, 2, 512], F32, tag="pt",
                                            name=f"pt{g}_{ob}")
                                   for ob in range(4)]
                            for ob in range(4 if not os.environ.get("KV2_NO_L1MM") else 0):
                                for tci in range(TC):
                                    for kb2 in range(KB):
                                        c0 = g4 * 512 + ob * 128
                                        st = wq1[:, kb2, c0:c0 + 128]
                                        nc.tensor.matmul(
                                            pts[ob][:, tci, :],
                                            st.unsqueeze(1).broadcast_to(
                                                [128, 2, 128]),
                                            xpair[:, kb2, :,
                                                  tci * 512:(tci + 1) * 512],
                                            start=(kb2 == 0),
                                            stop=(kb2 == KB - 1),
                                            perf_mode=PM.DoubleRow)
                            for ob in range(4):
                                hsl = hst.tile([128, T], I16, tag="hsl")
                                nc.scalar.activation(hsl[:], pts[ob][:],
                                                     ACTF.Relu,
                                                     scale=1.0 / 128.0)
                                nc.vector.tensor_tensor(macc[:], macc[:],
                                                        hsl[:], OP.max)
                                r0 = g * 512 + ob * 128
                                nc.sync.dma_start(h_hbm[r0:r0 + 128, :], hsl[:])

                # ---- Phase M: per-token scales --------------------------------
                with tc.tile_pool(name="ps_m", bufs=2, space="PSUM") as psm, \
                     tc.tile_pool(name="msc", bufs=1) as msc:
                    maccf = msc.tile([128, T], F32)
                    nc.vector.tensor_scalar_add(maccf[:], macc[:], 0.0)
                    M1 = msc.tile([128, TB], F32)
                    for tb in range(TB):
                        ptr = psm.tile([128, 128], F32, tag="trp")
                        nc.tensor.transpose(ptr[:],
                                            maccf[:, tb * 128:(tb + 1) * 128],
                                            ident[:])
                        nc.vector.tensor_reduce(M1[:, tb:tb + 1], ptr[:],
                                                axis=AX.X, op=OP.max)
                    hmax = msc.tile([128, TB], F32)
                    nc.vector.tensor_tensor(hmax[:], M1[:], C[:], OP.mult)
                    hmaxc = msc.tile([128, TB], F32)
                    nc.vector.tensor_scalar_max(hmaxc[:], hmax[:], EPS)
                    rch = msc.tile([128, TB], F32)
                    nc.vector.reciprocal(rch[:], hmaxc[:])
                    sh = msc.tile([128, TB], F32)
                    nc.vector.tensor_scalar_mul(sh[:], rch[:], 127.0)
                    g_tok = msc.tile([128, TB], F32)
                    nc.vector.tensor_tensor(g_tok[:], C[:], sh[:], OP.mult)
                    nc.vector.tensor_scalar(fscale[:], hmaxc[:], mu2_127,
                                            1.0 / 128.0, OP.mult, OP.mult)
                    ptg = psm.tile([TB, 128], F32, tag="ptg")
                    nc.tensor.transpose(ptg[:], g_tok[:], ident[:])
                    gsb = msc.tile([TB, 128], F32)
                    nc.scalar.copy(gsb[:], ptg[:])
                    g_dram = dram.tile([TB, 128], F32)
                    nc.sync.dma_start(g_dram[:], gsb[:])
                    g_row = msc.tile([1, T], F32)
                    nc.sync.dma_start(g_row[:],
                                      g_dram[:].rearrange("b t -> (b t)")[None, :])
                    for half in range(T // 512):
                        pg = psm.tile([128, 512], F32, tag="pg")
                        nc.tensor.matmul(pg[:], ones_row[:],
                                         g_row[:, half * 512:(half + 1) * 512],
                                         start=True, stop=True)
                        # clamp (pathological all-zero-token g) + fp16 convert
                        nc.vector.tensor_scalar(
                            G[:, half * 512:(half + 1) * 512], pg[:],
                            60000.0, None, OP.min)

            # ==== L2 era ====================================================
            with tc.tile_pool(name="l2big", bufs=1) as l2p:
                hpair = l2p.tile([128, OB, 2, T], E4)

                # Phase H: h int16 -> (hih16, loh) pair, 2-ob groups
                with tc.tile_pool(name="hio", bufs=2) as hp, \
                     tc.tile_pool(name="hgp", bufs=1) as hgp, \
                     tc.tile_pool(name="w2f", bufs=2) as w2f, \
                     tc.tile_pool(name="w2b", bufs=2) as w2b, \
                     tc.tile_pool(name="w2q", bufs=3) as w2qp, \
                     tc.tile_pool(name="ost", bufs=3) as ostp, \
                     tc.tile_pool(name="ps2", bufs=8, space="PSUM") as ps2:

                    def emit_h_group(grp):
                        # obs [4g, 4g+4): rows [512g, 512(g+1))
                        hread = hp.tile([128, 4, T], I16, tag="hread")
                        nc.sync.dma_start(
                            hread[:],
                            h_hbm[grp * 512:(grp + 1) * 512, :]
                            .rearrange("(ob p) t -> p ob t", p=128))
                        hg = hgp.tile([128, 4, T], mybir.dt.float16, tag="hg")
                        nc.vector.tensor_tensor(
                            hg[:], hread[:],
                            G[:].unsqueeze(1).broadcast_to([128, 4, T]),
                            OP.mult)
                        ob0 = grp * 4
                        nc.vector.tensor_scalar(
                            hpair[:, ob0:ob0 + 4, 0, :], hg[:],
                            MAGIC16, -MAGIC16, OP.add, OP.add)
                        nc.vector.tensor_scalar(hg[:], hg[:], MAGIC, -MAGIC,
                                                OP.add, OP.add)
                        nc.vector.tensor_tensor(
                            hpair[:, ob0:ob0 + 4, 1, :], hg[:],
                            hpair[:, ob0:ob0 + 4, 0, :], OP.subtract)

                    if os.environ.get("KV2_NO_H"):
                        def emit_h_group(grp):
                            return
                    for gg in range(4):
                        emit_h_group(gg)

                    for mg in range(4):
                        pts2 = [ps2.tile([128, 512], F32, tag="pt2",
                                         name=f"pt2_{mg}_{tb}")
                                for tb in range(TB)]
                        for grp in range(16):
                            if mg == 0 and grp <= 10 and grp % 2 == 0:
                                emit_h_group(grp + 4)
                                emit_h_group(grp + 5)
                            if os.environ.get("KV2_NO_W2"):
                                continue
                            wgf = w2f.tile([128, 4, 512], F32, tag="w2f")
                            nc.sync.dma_start(
                                wgf[:],
                                w2t_in[grp * 512:(grp + 1) * 512,
                                       mg * 512:(mg + 1) * 512]
                                .rearrange("(ob p) m -> p ob m", p=128))
                            u1 = w2b.tile([128, 4, 512], F32, tag="u1")
                            nc.scalar.activation(u1[:], wgf[:], ACTF.Copy,
                                                 bias=MAGIC, scale=s_w2)
                            nc.gpsimd.tensor_scalar(u1[:], u1[:],
                                                    MAGIC + 1.0, MAGIC - 1.0,
                                                    OP.min, OP.max)
                            wq2g = w2qp.tile([128, 4, 512], E4, tag="wq2")
                            if grp % 2 == 0:
                                nc.vector.tensor_scalar(wq2g[:], u1[:],
                                                        -MAGIC, 128.0,
                                                        OP.add, OP.mult)
                            else:
                                nc.scalar.activation(wq2g[:], u1[:],
                                                     ACTF.Copy,
                                                     bias=-MAGIC * 128.0,
                                                     scale=128.0)
                            for j in range(4 if not os.environ.get("KV2_NO_L2MM") else 0):
                                ob = grp * 4 + j
                                mv = wq2g[:, j, :].unsqueeze(1).broadcast_to(
                                    [128, 2, 512])
                                for tb in range(TB):
                                    nc.tensor.matmul(
                                        pts2[tb][:],
                                        hpair[:, ob, :,
                                              tb * 128:(tb + 1) * 128],
                                        mv,
                                        start=(ob == 0), stop=(ob == OB - 1),
                                        perf_mode=PM.DoubleRow)
                        for tb in range(TB if not os.environ.get("KV2_NO_L2MM") else 0):
                            osb = ostp.tile([128, 512], F32, tag="ostage")
                            nc.scalar.activation(osb[:], pts2[tb][:],
                                                 ACTF.Copy,
                                                 scale=fscale[:, tb:tb + 1])
                            nc.sync.dma_start(
                                y_out[tb * 128:(tb + 1) * 128,
                                      mg * 512:(mg + 1) * 512], osb[:])

    _split_excess_waits(nc)
    return nc


_NC = None


def kernel(x, w1, w2):
    global _NC
    if _NC is None:
        _NC = build_nc()
    x = np.ascontiguousarray(np.asarray(x, dtype=np.float32)).reshape(B * S, D)
    w1t = np.ascontiguousarray(np.asarray(w1, dtype=np.float32).T)  # [D, O]
    w2t = np.ascontiguousarray(np.asarray(w2, dtype=np.float32).T)  # [O, D]
    in_maps = []
    for i in range(NCORES):
        in_maps.append({
            "x": x[i * T:(i + 1) * T],
            "w1t": w1t,
            "w2t": w2t,
            "wsh1": w1t[i * 256:(i + 1) * 256],
            "wsh2": w2t[i * 1024:(i + 1) * 1024],
        })
    res = run_bass_kernel_spmd(_NC, in_maps, core_ids=list(range(NCORES)))
    out = np.concatenate([res.results[i]["out"] for i in range(NCORES)], axis=0)
    return out.reshape(B, S, D)
